# revision 12
# baseline (speedup 1.0000x reference)
"""
MultiHeadAttention forward for Trainium2, 8 NeuronCores.

Sharding across the 8 cores: core c handles batch b = c//4 and the 4
heads h0 = (c%4)*4 .. h0+4 (an E=256 slice of the projection dim).
w_q/w_k/w_v are row-sliced, w_o is column-sliced (Megatron), so each
core emits its attn block plus a partial `out` that the host sums over
the 4 cores sharing a batch.

All inputs are pre-transposed on the host so the kernel never has to
transpose its DRAM inputs:
    qT/kT/vT : [D, S]   (x[b].T)
    wqT/wkT/wvT : [D, E]   (w[e0:e0+E, :].T)  - matmul lhsT for projections
    woT : [E, D]           (w_o[:, e0:e0+E].T) - matmul rhs for out proj

All matmuls run as float32r (single-pass fp32 on the PE, ~4x faster
than the fp32 LOW_HIGH double-pass) with fp32 PSUM accumulation.

Per-core pipeline:
  stage 0: QT[e,s], KT[e,s] (head pairs stacked on the 128 partitions)
           and V[s,e] via PE matmuls.
  stage 1, per head:
    phase A (attn output + denominators), per 128-row tile i:
        S_i = QhT_i.T @ KhT   (PE, K=64, two 2-bank psum halves)
        P_i = exp(S_i/8) with fused row-sum accumulators (ACT)
        invd = 1/den (DVE) ; A_i = P_i * invd in place (DVE, 2x mode)
        DMA A_i -> attn[h, i]
    phase B (context, transposed orientation), per 128-row tile j:
        St_j = KhT_j.T @ QhT  (PE)  ;  Pt_j = exp(St_j/8)  (ACT)
        Ct[hd, :] += Vh_j.T @ Pt_j  (PE, psum accumulate over j)
    C-normalize: invd columns -> PE-transposed into a [1, S] row,
        outer-product broadcast to [64, S], Ct * bcast on DVE.
  stage 2: out = sum over head pairs  Ct_hp.T @ woT_hp -> partial out
"""

import numpy as np

import concourse.bass as bass
import concourse.mybir as mybir
import concourse.tile as tile
from concourse import bacc
from concourse.masks import make_identity

F32 = mybir.dt.float32
F32R = mybir.dt.float32r
BF16 = mybir.dt.bfloat16
MM_DT = F32R  # dtype of all matmul operands (BF16 or F32R)

# Problem shapes (hardcoded per contract)
B, S, D, H = 2, 2048, 1024, 16
HD = D // H                    # 64
N_CORES = 8
H_LOC = H * B // N_CORES       # 4 heads per core
E = H_LOC * HD                 # 256 projection cols per core
P = 128

SCALE = 1.0 / float(np.sqrt(HD))


def _ceil_div(a, b):
    return (a + b - 1) // b


def build_nc(s=S, d=D, mm_dt=MM_DT):
    """Build the single-core Bass program (SPMD across 8 cores)."""
    nt = s // P                  # 128-row tiles of the sequence
    dt_n = d // P                # contraction tiles for projections
    n_ch = _ceil_div(s, 512)     # 512-wide column chunks of the seq dim
    half = min(s, 1024)          # psum half-tile width for scores
    n_hh = _ceil_div(s, half)
    hch = _ceil_div(half, 512)   # 512-chunks per half

    nc = bacc.Bacc("TRN2", target_bir_lowering=False, debug=False)

    use_heat = mm_dt == F32R

    def mm(out, lhsT, rhs, heat=False, **kw):
        if heat and use_heat:
            lhsT = lhsT.bitcast(F32)
            rhs = rhs.bitcast(F32)
        nc.tensor.matmul(out, lhsT, rhs, **kw)

    qT = nc.dram_tensor("qT", [d, s], mm_dt, kind="ExternalInput")
    kT = nc.dram_tensor("kT", [d, s], mm_dt, kind="ExternalInput")
    vT = nc.dram_tensor("vT", [d, s], mm_dt, kind="ExternalInput")
    wqT = nc.dram_tensor("wqT", [d, E], mm_dt, kind="ExternalInput")
    wkT = nc.dram_tensor("wkT", [d, E], mm_dt, kind="ExternalInput")
    wvT = nc.dram_tensor("wvT", [d, E], mm_dt, kind="ExternalInput")
    woT = nc.dram_tensor("woT", [E, d], mm_dt, kind="ExternalInput")

    attn = nc.dram_tensor("attn", [H_LOC, s, s], F32, kind="ExternalOutput")
    outp = nc.dram_tensor("outp", [s, d], F32, kind="ExternalOutput")

    with tile.TileContext(nc) as tc:
        with (
            tc.tile_pool(name="consts", bufs=1) as consts,
            tc.tile_pool(name="weights", bufs=1) as wpool,
            tc.tile_pool(name="persist", bufs=1) as persist,
            tc.tile_pool(name="io", bufs=2) as io,
            tc.tile_pool(name="work", bufs=2) as work,
            tc.tile_pool(name="stats", bufs=4) as stats,
        ):
            ident = consts.tile([P, P], F32, name="ident")
            make_identity(nc, ident)
            ones_row = consts.tile([1, HD], F32, name="ones_row")
            nc.vector.memset(ones_row, 1.0)
            hconst = consts.tile([P, 1], BF16, name="hconst")
            nc.vector.memset(hconst, 1.0)

            def heat_pe():
                # dummy bf16 LDWEIGHTS: registers as counted PE activity so
                # the HAM clock gate stays at 2.4 GHz (f32r HIGH-mode
                # matmuls don't count); clobbers only array column 0,
                # which every self-loading matmul re-loads anyway.
                nc.tensor.ldweights(hconst)

            # ---- weights to SBUF ----
            wq_sb = wpool.tile([P, dt_n, E], mm_dt, name="wq_sb")
            wk_sb = wpool.tile([P, dt_n, E], mm_dt, name="wk_sb")
            wv_sb = wpool.tile([P, dt_n, E], mm_dt, name="wv_sb")
            wo_sb = wpool.tile([P, E // P, d], mm_dt, name="wo_sb")
            nc.sync.dma_start(wq_sb, wqT.ap().rearrange("(t p) e -> p t e", p=P))
            nc.sync.dma_start(wk_sb, wkT.ap().rearrange("(t p) e -> p t e", p=P))
            nc.sync.dma_start(wv_sb, wvT.ap().rearrange("(t p) e -> p t e", p=P))
            nc.sync.dma_start(wo_sb, woT.ap().rearrange("(t p) f -> p t f", p=P))

            # ---- persistent activations ----
            QT = persist.tile([P, E // P, s], mm_dt, name="QT")
            KT = persist.tile([P, E // P, s], mm_dt, name="KT")
            V_sb = persist.tile([P, nt, E], mm_dt, name="V_sb")
            Ct = persist.tile([P, E // P, s], mm_dt, name="Ct")

            # ================= stage 0: projections =================
            with tc.tile_pool(name="psum0", bufs=2, space="PSUM") as psum0:
                for (src, w_sb, dst) in ((qT, wq_sb, QT), (kT, wk_sb, KT)):
                    ps = [
                        psum0.tile([P, s], F32, name=f"proj_ps{et}", tag="proj")
                        for et in range(E // P)
                    ]
                    for dti in range(dt_n):
                        xin = io.tile([P, s], mm_dt, name="xin", tag="xin")
                        nc.sync.dma_start(
                            xin, src.ap()[dti * P : (dti + 1) * P, :]
                        )
                        heat_pe()
                        for et in range(E // P):
                            for c in range(n_ch):
                                sl = slice(c * 512, min(s, (c + 1) * 512))
                                mm(
                                    ps[et][:, sl],
                                    w_sb[:, dti, et * P : (et + 1) * P],
                                    xin[:, sl],
                                    heat=(et == 0 and c == 0),
                                    start=(dti == 0),
                                    stop=(dti == dt_n - 1),
                                )
                    for et in range(E // P):
                        nc.vector.tensor_copy(dst[:, et, :], ps[et])

                # V natural: one s-tile per 512-wide psum bank
                n_vg = _ceil_div(nt, 4)
                for g in range(n_vg):
                    t0 = g * 4
                    tn = min(4, nt - t0)
                    vps = psum0.tile([P, tn * 512], F32, name="vps", tag="proj")
                    for dti in range(dt_n):
                        xin = io.tile([P, s], mm_dt, name="xin", tag="xin")
                        nc.sync.dma_start(
                            xin, vT.ap()[dti * P : (dti + 1) * P, :]
                        )
                        for t in range(tn):
                            st = t0 + t
                            mm(
                                vps[:, t * 512 : t * 512 + E],
                                xin[:, st * P : (st + 1) * P],
                                wv_sb[:, dti, :],
                                heat=(t == 0),
                                start=(dti == 0),
                                stop=(dti == dt_n - 1),
                            )
                    for t in range(tn):
                        nc.vector.tensor_copy(
                            V_sb[:, t0 + t, :], vps[:, t * 512 : t * 512 + E]
                        )

            # ================= stage 1: attention =================
            with tc.tile_pool(name="psum1", bufs=1, space="PSUM") as psum1:
                for h in range(H_LOC):
                    eoff = (h % 2) * HD
                    hp = h // 2
                    qh = QT[eoff : eoff + HD, hp, :]
                    kh = KT[eoff : eoff + HD, hp, :]
                    invd_all = stats.tile(
                        [P, nt], F32, name="invd_all", tag="invd_all", bufs=2
                    )

                    # --- phase A: normalized attn rows + denominators ---
                    for it in range(nt):
                        a_i = work.tile([P, s], F32, name="a_i", tag="A", bufs=2)
                        dens = []
                        for hh in range(n_hh):
                            w = min(half, s - hh * half)
                            sp = psum1.tile(
                                [P, half], F32, name="sp", tag="half", bufs=2
                            )
                            for c in range(hch):
                                lo = hh * half + c * 512
                                if lo >= s:
                                    break
                                wc = min(512, s - lo)
                                mm(
                                    sp[:, c * 512 : c * 512 + wc],
                                    qh[:, it * P : (it + 1) * P],
                                    kh[:, lo : lo + wc],
                                    heat=(hh == 0 and c == 0),
                                    start=True,
                                    stop=True,
                                )
                            den_h = stats.tile(
                                [P, 1], F32, name="den_h", tag="den"
                            )
                            nc.scalar.activation(
                                a_i[:, hh * half : hh * half + w],
                                sp[:, :w],
                                mybir.ActivationFunctionType.Exp,
                                scale=SCALE,
                                accum_out=den_h,
                            )
                            dens.append(den_h)
                        heat_pe()
                        den = dens[0]
                        for dh in dens[1:]:
                            den2 = stats.tile([P, 1], F32, name="den2", tag="den")
                            nc.vector.tensor_tensor(
                                den2, den, dh, mybir.AluOpType.add
                            )
                            den = den2
                        nc.vector.reciprocal(invd_all[:, it : it + 1], den)
                        nc.vector.tensor_scalar_mul(
                            a_i, a_i, invd_all[:, it : it + 1]
                        )
                        nc.sync.dma_start(
                            attn.ap()[h, it * P : (it + 1) * P, :], a_i
                        )

                    # --- phase B: context in transposed orientation ---
                    ct_ps = psum1.tile([HD, s], F32, name="ct_ps", tag="ct")
                    for jt in range(nt):
                        heat_pe()
                        pt = work.tile([P, s], mm_dt, name="pt", tag="PT", bufs=2)
                        for hh in range(n_hh):
                            w = min(half, s - hh * half)
                            sp = psum1.tile(
                                [P, half], F32, name="sp", tag="half", bufs=2
                            )
                            for c in range(hch):
                                lo = hh * half + c * 512
                                if lo >= s:
                                    break
                                wc = min(512, s - lo)
                                mm(
                                    sp[:, c * 512 : c * 512 + wc],
                                    kh[:, jt * P : (jt + 1) * P],
                                    qh[:, lo : lo + wc],
                                    heat=(hh == 0 and c == 0),
                                    start=True,
                                    stop=True,
                                )
                            nc.scalar.activation(
                                pt[:, hh * half : hh * half + w],
                                sp[:, :w],
                                mybir.ActivationFunctionType.Exp,
                                scale=SCALE,
                            )
                        for c in range(n_ch):
                            sl = slice(c * 512, min(s, (c + 1) * 512))
                            mm(
                                ct_ps[:, sl],
                                V_sb[:, jt, h * HD : (h + 1) * HD],
                                pt[:, sl],
                                start=(jt == 0),
                                stop=(jt == nt - 1),
                            )

                    # --- C-normalize: Ct * (1/den) broadcast along i ---
                    ctu = work.tile([HD, s], F32, name="ctu", tag="CTU", bufs=2)
                    nc.vector.tensor_copy(ctu, ct_ps)
                    row_ps = psum1.tile([1, s], F32, name="row_ps", tag="ct")
                    for it in range(nt):
                        nc.tensor.matmul(
                            row_ps[0:1, it * P : (it + 1) * P],
                            lhsT=invd_all[:, it : it + 1],
                            rhs=ident,
                            is_transpose=True,
                            start=(it % 4 == 0),
                            stop=(it % 4 == 3 or it == nt - 1),
                        )
                    row_sb = stats.tile([1, s], F32, name="row_sb", tag="rowsb", bufs=1)
                    nc.vector.tensor_copy(row_sb, row_ps)
                    bc_ps = psum1.tile([HD, s], F32, name="bc_ps", tag="ct")
                    for c in range(n_ch):
                        sl = slice(c * 512, min(s, (c + 1) * 512))
                        nc.tensor.matmul(
                            bc_ps[:, sl],
                            ones_row,
                            row_sb[0:1, sl],
                            start=True,
                            stop=True,
                        )
                    bc_sb = work.tile([HD, s], F32, name="bc_sb", tag="CTU", bufs=2)
                    nc.vector.tensor_copy(bc_sb, bc_ps)
                    nc.vector.tensor_tensor(
                        Ct[eoff : eoff + HD, hp, :],
                        ctu,
                        bc_sb,
                        mybir.AluOpType.mult,
                    )

            # ================= stage 2: output projection =================
            with tc.tile_pool(name="psum2", bufs=2, space="PSUM") as psum2:
                for it in range(nt):
                    o_ps = psum2.tile([P, d], F32, name="o_ps", tag="out")
                    heat_pe()
                    for hpi in range(E // P):
                        for c in range(_ceil_div(d, 512)):
                            sl = slice(c * 512, min(d, (c + 1) * 512))
                            mm(
                                o_ps[:, sl],
                                Ct[:, hpi, it * P : (it + 1) * P],
                                wo_sb[:, hpi, sl],
                                heat=(hpi == 0 and c == 0),
                                start=(hpi == 0),
                                stop=(hpi == E // P - 1),
                            )
                    o_sb = work.tile([P, d], F32, name="o_sb", tag="O", bufs=2)
                    nc.vector.tensor_copy(o_sb, o_ps)
                    nc.sync.dma_start(outp.ap()[it * P : (it + 1) * P, :], o_sb)

    nc.compile()
    return nc


_NC_CACHE = {}


def _get_nc():
    if "nc" not in _NC_CACHE:
        _NC_CACHE["nc"] = build_nc()
    return _NC_CACHE["nc"]


def make_in_maps(q, k, v, w_q, w_k, w_v, w_o):
    import ml_dtypes

    in_np = ml_dtypes.bfloat16 if MM_DT == BF16 else np.float32
    q = np.asarray(q, np.float32)
    k = np.asarray(k, np.float32)
    v = np.asarray(v, np.float32)
    w_q = np.asarray(w_q, np.float32)
    w_k = np.asarray(w_k, np.float32)
    w_v = np.asarray(w_v, np.float32)
    w_o = np.asarray(w_o, np.float32)

    qT = [np.ascontiguousarray(q[b].T) for b in range(B)]
    kT = [np.ascontiguousarray(k[b].T) for b in range(B)]
    vT = [np.ascontiguousarray(v[b].T) for b in range(B)]

    in_maps = []
    for c in range(N_CORES):
        b = c // (N_CORES // B)
        e0 = (c % (N_CORES // B)) * E
        in_maps.append(
            {
                "qT": qT[b],
                "kT": kT[b],
                "vT": vT[b],
                "wqT": np.ascontiguousarray(w_q[e0 : e0 + E, :].T),
                "wkT": np.ascontiguousarray(w_k[e0 : e0 + E, :].T),
                "wvT": np.ascontiguousarray(w_v[e0 : e0 + E, :].T),
                "woT": np.ascontiguousarray(w_o[:, e0 : e0 + E].T),
            }
        )
    if in_np is not np.float32:
        in_maps = [
            {k2: np.ascontiguousarray(v2.astype(in_np)) for k2, v2 in m.items()}
            for m in in_maps
        ]
    return in_maps


def gather_outputs(results):
    attn = np.empty((B, H, S, S), np.float32)
    out = np.zeros((B, S, D), np.float32)
    for c in range(N_CORES):
        b = c // (N_CORES // B)
        h0 = (c % (N_CORES // B)) * H_LOC
        attn[b, h0 : h0 + H_LOC] = results[c]["attn"]
        out[b] += results[c]["outp"]
    return out, attn


def kernel(q, k, v, w_q, w_k, w_v, w_o, _trace=False, _tmpdir=None):
    from concourse.bass_utils import run_bass_kernel_spmd

    nc = _get_nc()
    in_maps = make_in_maps(q, k, v, w_q, w_k, w_v, w_o)
    res = run_bass_kernel_spmd(
        nc,
        in_maps,
        core_ids=list(range(N_CORES)),
        trace=_trace,
        tmpdir=_tmpdir,
    )
    out, attn = gather_outputs(res.results)
    kernel.last_results = res
    return out, attn


# revision 19
# speedup vs baseline: 1.1959x; 1.1959x over previous
"""
MultiHeadAttention forward for Trainium2, 8 NeuronCores.

Sharding across the 8 cores: core c handles batch b = c//4 and the 4
heads h0 = (c%4)*4 .. h0+4 (an E=256 slice of the projection dim).
w_q/w_k/w_v are row-sliced, w_o is column-sliced (Megatron), so each
core emits its attn block plus a partial `out` that the host sums over
the 4 cores sharing a batch.

All inputs are pre-transposed on the host so the kernel never has to
transpose its DRAM inputs:
    qT/kT/vT : [D, S]   (x[b].T)
    wqT/wkT/wvT : [D, E]   (w[e0:e0+E, :].T)  - matmul lhsT for projections
    woT : [E, D]           (w_o[:, e0:e0+E].T) - matmul rhs for out proj

All matmuls run as float32r (single-pass fp32 on the PE, ~4x faster
than the fp32 LOW_HIGH double-pass) with fp32 PSUM accumulation.

Per-core pipeline:
  stage 0: QT[e,s], KT[e,s] (head pairs stacked on the 128 partitions)
           and V[s,e] via PE matmuls.
  stage 1, per head:
    phase A (attn output + denominators), per 128-row tile i:
        S_i = QhT_i.T @ KhT   (PE, K=64, two 2-bank psum halves)
        P_i = exp(S_i/8) with fused row-sum accumulators (ACT)
        invd = 1/den (DVE) ; A_i = P_i * invd in place (DVE, 2x mode)
        DMA A_i -> attn[h, i]
    phase B (context, transposed orientation), per 128-row tile j:
        St_j = KhT_j.T @ QhT  (PE)  ;  Pt_j = exp(St_j/8)  (ACT)
        Ct[hd, :] += Vh_j.T @ Pt_j  (PE, psum accumulate over j)
    C-normalize: invd columns -> PE-transposed into a [1, S] row,
        outer-product broadcast to [64, S], Ct * bcast on DVE.
  stage 2: out = sum over head pairs  Ct_hp.T @ woT_hp -> partial out
"""

import numpy as np

import concourse.bass as bass
import concourse.mybir as mybir
import concourse.tile as tile
from concourse import bacc
from concourse.masks import make_identity

F32 = mybir.dt.float32
F32R = mybir.dt.float32r
BF16 = mybir.dt.bfloat16
MM_DT = F32R  # dtype of all matmul operands (BF16 or F32R)

# Problem shapes (hardcoded per contract)
B, S, D, H = 2, 2048, 1024, 16
HD = D // H                    # 64
N_CORES = 8
H_LOC = H * B // N_CORES       # 4 heads per core
E = H_LOC * HD                 # 256 projection cols per core
P = 128

SCALE = 1.0 / float(np.sqrt(HD))


def _ceil_div(a, b):
    return (a + b - 1) // b


def build_nc(s=S, d=D, mm_dt=MM_DT):
    """Build the single-core Bass program (SPMD across 8 cores)."""
    nt = s // P                  # 128-row tiles of the sequence
    dt_n = d // P                # contraction tiles for projections
    n_ch = _ceil_div(s, 512)     # 512-wide column chunks of the seq dim
    half = min(s, 1024)          # psum half-tile width for scores
    n_hh = _ceil_div(s, half)
    hch = _ceil_div(half, 512)   # 512-chunks per half

    nc = bacc.Bacc("TRN2", target_bir_lowering=False, debug=False)

    def mm(out, lhsT, rhs, heat=False, **kw):
        nc.tensor.matmul(out, lhsT, rhs, **kw)

    qT = nc.dram_tensor("qT", [d, s], mm_dt, kind="ExternalInput")
    kT = nc.dram_tensor("kT", [d, s], mm_dt, kind="ExternalInput")
    vT = nc.dram_tensor("vT", [d, s], mm_dt, kind="ExternalInput")
    wqT = nc.dram_tensor("wqT", [d, E], mm_dt, kind="ExternalInput")
    wkT = nc.dram_tensor("wkT", [d, E], mm_dt, kind="ExternalInput")
    wvT = nc.dram_tensor("wvT", [d, E], mm_dt, kind="ExternalInput")
    woT = nc.dram_tensor("woT", [E, d], mm_dt, kind="ExternalInput")

    attn = nc.dram_tensor("attn", [H_LOC, s, s], F32, kind="ExternalOutput")
    outp = nc.dram_tensor("outp", [s, d], F32, kind="ExternalOutput")

    with tile.TileContext(nc) as tc:
        with (
            tc.tile_pool(name="consts", bufs=1) as consts,
            tc.tile_pool(name="weights", bufs=1) as wpool,
            tc.tile_pool(name="persist", bufs=1) as persist,
            tc.tile_pool(name="work", bufs=2) as work,
            tc.tile_pool(name="stats", bufs=4) as stats,
        ):
            ident = consts.tile([P, P], F32, name="ident")
            make_identity(nc, ident)
            ones_f32 = consts.tile([1, HD], F32, name="ones_f32")
            nc.vector.memset(ones_f32, 1.0)
            ones_row = consts.tile([1, HD], mm_dt, name="ones_row")
            nc.vector.tensor_copy(ones_row, ones_f32)

            # ---- weights to SBUF ----
            wq_sb = wpool.tile([P, dt_n, E], mm_dt, name="wq_sb")
            wk_sb = wpool.tile([P, dt_n, E], mm_dt, name="wk_sb")
            wv_sb = wpool.tile([P, dt_n, E], mm_dt, name="wv_sb")
            wo_sb = wpool.tile([P, E // P, d], mm_dt, name="wo_sb")
            nc.sync.dma_start(wq_sb, wqT.ap().rearrange("(t p) e -> p t e", p=P))
            nc.sync.dma_start(wk_sb, wkT.ap().rearrange("(t p) e -> p t e", p=P))
            nc.sync.dma_start(wv_sb, wvT.ap().rearrange("(t p) e -> p t e", p=P))
            nc.sync.dma_start(wo_sb, woT.ap().rearrange("(t p) f -> p t f", p=P))

            # ---- persistent activations ----
            QT = persist.tile([P, E // P, s], mm_dt, name="QT")
            KT = persist.tile([P, E // P, s], mm_dt, name="KT")
            V_sb = persist.tile([P, nt, E], mm_dt, name="V_sb")
            Ct = persist.tile([P, E // P, s], mm_dt, name="Ct")

            # ================= stage 0: projections =================
            with tc.tile_pool(name="psum0", bufs=2, space="PSUM") as psum0:
                for (src, w_sb, dst) in ((qT, wq_sb, QT), (kT, wk_sb, KT)):
                    ps = [
                        psum0.tile([P, s], F32, name=f"proj_ps{et}", tag="proj")
                        for et in range(E // P)
                    ]
                    for dti in range(dt_n):
                        xin = work.tile([P, s], mm_dt, name="xin", tag="xin", bufs=2)
                        nc.sync.dma_start(
                            xin, src.ap()[dti * P : (dti + 1) * P, :]
                        )
                        for et in range(E // P):
                            for c in range(n_ch):
                                sl = slice(c * 512, min(s, (c + 1) * 512))
                                mm(
                                    ps[et][:, sl],
                                    w_sb[:, dti, et * P : (et + 1) * P],
                                    xin[:, sl],
                                    heat=(et == 0 and c == 0),
                                    start=(dti == 0),
                                    stop=(dti == dt_n - 1),
                                )
                    for et in range(E // P):
                        nc.vector.tensor_copy(dst[:, et, :], ps[et])

                # V natural: one s-tile per 512-wide psum bank
                n_vg = _ceil_div(nt, 4)
                for g in range(n_vg):
                    t0 = g * 4
                    tn = min(4, nt - t0)
                    vps = psum0.tile([P, tn * 512], F32, name="vps", tag="proj")
                    for dti in range(dt_n):
                        xin = work.tile([P, s], mm_dt, name="xin", tag="xin", bufs=2)
                        nc.sync.dma_start(
                            xin, vT.ap()[dti * P : (dti + 1) * P, :]
                        )
                        for t in range(tn):
                            st = t0 + t
                            mm(
                                vps[:, t * 512 : t * 512 + E],
                                xin[:, st * P : (st + 1) * P],
                                wv_sb[:, dti, :],
                                heat=(t == 0),
                                start=(dti == 0),
                                stop=(dti == dt_n - 1),
                            )
                    for t in range(tn):
                        nc.vector.tensor_copy(
                            V_sb[:, t0 + t, :], vps[:, t * 512 : t * 512 + E]
                        )

            # ================= stage 1: attention =================
            # Heads are processed in pairs (2*hp, 2*hp+1), stacked on
            # partitions 0-63 / 64-127 of the QT/KT head-pair tiles. The
            # K=64 score matmuls of a pair land on PE row-groups 0/64 and
            # the M=64 ctx matmuls on column-groups 0/64 (tile_position is
            # derived from the AP base partitions), so each pair executes
            # concurrently on the PE array.
            for hp in range(E // P):
                qh = [QT[x * HD : (x + 1) * HD, hp, :] for x in range(2)]
                kh = [KT[x * HD : (x + 1) * HD, hp, :] for x in range(2)]
                invd = [
                    stats.tile(
                        [P, nt], F32, name=f"invd{x}", tag=f"invd{x}", bufs=2
                    )
                    for x in range(2)
                ]

                # --- phase A: normalized attn rows + denominators ---
                with tc.tile_pool(
                    name=f"psA{hp}", bufs=4, space="PSUM"
                ) as psA:
                    for it in range(nt):
                        a_t = [
                            work.tile([P, s], F32, name=f"a_{x}", tag="A", bufs=3)
                            for x in range(2)
                        ]
                        dens = [
                            stats.tile(
                                [P, max(n_hh, 2)], F32, name=f"dens{x}", tag="dens"
                            )
                            for x in range(2)
                        ]
                        for hh in range(n_hh):
                            w = min(half, s - hh * half)
                            sp2 = [
                                psA.tile(
                                    [P, half], F32, name=f"spA{x}", tag="halfA"
                                )
                                for x in range(2)
                            ]
                            for c in range(hch):
                                lo = hh * half + c * 512
                                if lo >= s:
                                    break
                                wc = min(512, s - lo)
                                for x in range(2):
                                    mm(
                                        sp2[x][:, c * 512 : c * 512 + wc],
                                        qh[x][:, it * P : (it + 1) * P],
                                        kh[x][:, lo : lo + wc],
                                        start=True,
                                        stop=True,
                                    )
                            for x in range(2):
                                nc.scalar.activation(
                                    a_t[x][:, hh * half : hh * half + w],
                                    sp2[x][:, :w],
                                    mybir.ActivationFunctionType.Exp,
                                    scale=SCALE,
                                    accum_out=dens[x][:, hh : hh + 1],
                                )
                        for x in range(2):
                            if n_hh > 1:
                                den = stats.tile([P, 1], F32, name="den", tag="den")
                                nc.vector.reduce_sum(
                                    den,
                                    dens[x][:, :n_hh],
                                    axis=mybir.AxisListType.X,
                                )
                            else:
                                den = dens[x][:, 0:1]
                            nc.vector.reciprocal(invd[x][:, it : it + 1], den)
                            nc.vector.tensor_scalar_mul(
                                a_t[x], a_t[x], invd[x][:, it : it + 1]
                            )
                            nc.sync.dma_start(
                                attn.ap()[2 * hp + x, it * P : (it + 1) * P, :],
                                a_t[x],
                            )

                # --- phase B: context in transposed orientation ---
                # ctx psum accumulators live at base partition 0 (matmul
                # dst at base 64 fails the ISA dst-partition check), so
                # the two heads get separate [64, 1024] tiles and the i
                # range is processed in two halves.
                n_ih = _ceil_div(s, 1024)
                ihw_all = min(s, 1024)
                ctu2 = [
                    work.tile([HD, s], F32, name=f"ctu{x}", tag="CTU", bufs=3)
                    for x in range(2)
                ]
                with tc.tile_pool(
                    name=f"psB{hp}", bufs=1, space="PSUM"
                ) as psB:
                    for ih in range(n_ih):
                        ilo = ih * 1024
                        ihw = min(1024, s - ilo)
                        cts = [
                            psB.tile(
                                [HD, ihw_all], F32, name=f"ct{x}",
                                tag=f"ct{x}", bufs=1,
                            )
                            for x in range(2)
                        ]
                        for jt in range(nt):
                            pt2 = [
                                work.tile(
                                    [P, ihw_all], mm_dt, name=f"pt{x}",
                                    tag="PT", bufs=3,
                                )
                                for x in range(2)
                            ]
                            for cc in range(_ceil_div(ihw, 512)):
                                lo = ilo + cc * 512
                                wc = min(512, s - lo)
                                spB = [
                                    psB.tile(
                                        [P, 512], F32, name=f"spB{x}",
                                        tag="halfB", bufs=4,
                                    )
                                    for x in range(2)
                                ]
                                for x in range(2):
                                    mm(
                                        spB[x][:, :wc],
                                        kh[x][:, jt * P : (jt + 1) * P],
                                        qh[x][:, lo : lo + wc],
                                        start=True,
                                        stop=True,
                                    )
                                for x in range(2):
                                    nc.scalar.activation(
                                        pt2[x][:, cc * 512 : cc * 512 + wc],
                                        spB[x][:, :wc],
                                        mybir.ActivationFunctionType.Exp,
                                        scale=SCALE,
                                    )
                            for cc in range(_ceil_div(ihw, 512)):
                                sl = slice(cc * 512, min(ihw, (cc + 1) * 512))
                                for x in range(2):
                                    mm(
                                        cts[x][:, sl],
                                        V_sb[:, jt, (2 * hp + x) * HD
                                             : (2 * hp + x + 1) * HD],
                                        pt2[x][:, sl],
                                        start=(jt == 0),
                                        stop=(jt == nt - 1),
                                    )
                        for x in range(2):
                            nc.vector.tensor_copy(
                                ctu2[x][:, ilo : ilo + ihw], cts[x][:, :ihw]
                            )

                    # --- C-normalize: Ct * (1/den) broadcast along i ---
                    for x in range(2):
                        row_sb = stats.tile(
                            [1, s], mm_dt, name=f"row_sb{x}", tag="rowsb",
                            bufs=1,
                        )
                        for ih in range(n_ih):
                            ilo = ih * 1024
                            ihw = min(1024, s - ilo)
                            row_ps = psB.tile(
                                [1, ihw_all], F32, name="row_ps", tag="ct0"
                            )
                            t_n = ihw // P
                            for t in range(t_n):
                                it = ih * (1024 // P) + t
                                nc.tensor.matmul(
                                    row_ps[0:1, t * P : (t + 1) * P],
                                    lhsT=invd[x][:, it : it + 1],
                                    rhs=ident,
                                    is_transpose=True,
                                    start=(t % 4 == 0),
                                    stop=(t % 4 == 3 or t == t_n - 1),
                                )
                            nc.vector.tensor_copy(
                                row_sb[0:1, ilo : ilo + ihw], row_ps[0:1, :ihw]
                            )
                        bc_sb = work.tile(
                            [HD, s], F32, name="bc_sb", tag="CTU", bufs=3
                        )
                        for ih in range(n_ih):
                            ilo = ih * 1024
                            ihw = min(1024, s - ilo)
                            bc_ps = psB.tile(
                                [HD, ihw_all], F32, name="bc_ps", tag="ct1"
                            )
                            for c in range(_ceil_div(ihw, 512)):
                                sl = slice(c * 512, min(ihw, (c + 1) * 512))
                                lo = ilo + c * 512
                                wc = min(512, s - lo)
                                mm(
                                    bc_ps[:, sl],
                                    ones_row,
                                    row_sb[0:1, lo : lo + wc],
                                    start=True,
                                    stop=True,
                                )
                            nc.vector.tensor_copy(
                                bc_sb[:, ilo : ilo + ihw], bc_ps[:, :ihw]
                            )
                        nc.vector.tensor_tensor(
                            Ct[x * HD : (x + 1) * HD, hp, :],
                            ctu2[x],
                            bc_sb,
                            mybir.AluOpType.mult,
                        )

            # ================= stage 2: output projection =================
            with tc.tile_pool(name="psum2", bufs=2, space="PSUM") as psum2:
                for it in range(nt):
                    o_ps = psum2.tile([P, d], F32, name="o_ps", tag="out")
                    for hpi in range(E // P):
                        for c in range(_ceil_div(d, 512)):
                            sl = slice(c * 512, min(d, (c + 1) * 512))
                            mm(
                                o_ps[:, sl],
                                Ct[:, hpi, it * P : (it + 1) * P],
                                wo_sb[:, hpi, sl],
                                heat=(hpi == 0 and c == 0),
                                start=(hpi == 0),
                                stop=(hpi == E // P - 1),
                            )
                    o_sb = work.tile([P, d], F32, name="o_sb", tag="O", bufs=2)
                    nc.vector.tensor_copy(o_sb, o_ps)
                    nc.sync.dma_start(outp.ap()[it * P : (it + 1) * P, :], o_sb)

    nc.compile()
    return nc


_NC_CACHE = {}


def _get_nc():
    if "nc" not in _NC_CACHE:
        _NC_CACHE["nc"] = build_nc()
    return _NC_CACHE["nc"]


def make_in_maps(q, k, v, w_q, w_k, w_v, w_o):
    import ml_dtypes

    in_np = ml_dtypes.bfloat16 if MM_DT == BF16 else np.float32
    q = np.asarray(q, np.float32)
    k = np.asarray(k, np.float32)
    v = np.asarray(v, np.float32)
    w_q = np.asarray(w_q, np.float32)
    w_k = np.asarray(w_k, np.float32)
    w_v = np.asarray(w_v, np.float32)
    w_o = np.asarray(w_o, np.float32)

    qT = [np.ascontiguousarray(q[b].T) for b in range(B)]
    kT = [np.ascontiguousarray(k[b].T) for b in range(B)]
    vT = [np.ascontiguousarray(v[b].T) for b in range(B)]

    in_maps = []
    for c in range(N_CORES):
        b = c // (N_CORES // B)
        e0 = (c % (N_CORES // B)) * E
        in_maps.append(
            {
                "qT": qT[b],
                "kT": kT[b],
                "vT": vT[b],
                "wqT": np.ascontiguousarray(w_q[e0 : e0 + E, :].T),
                "wkT": np.ascontiguousarray(w_k[e0 : e0 + E, :].T),
                "wvT": np.ascontiguousarray(w_v[e0 : e0 + E, :].T),
                "woT": np.ascontiguousarray(w_o[:, e0 : e0 + E].T),
            }
        )
    if in_np is not np.float32:
        in_maps = [
            {k2: np.ascontiguousarray(v2.astype(in_np)) for k2, v2 in m.items()}
            for m in in_maps
        ]
    return in_maps


def gather_outputs(results):
    attn = np.empty((B, H, S, S), np.float32)
    out = np.zeros((B, S, D), np.float32)
    for c in range(N_CORES):
        b = c // (N_CORES // B)
        h0 = (c % (N_CORES // B)) * H_LOC
        attn[b, h0 : h0 + H_LOC] = results[c]["attn"]
        out[b] += results[c]["outp"]
    return out, attn


def kernel(q, k, v, w_q, w_k, w_v, w_o, _trace=False, _tmpdir=None):
    from concourse.bass_utils import run_bass_kernel_spmd

    nc = _get_nc()
    in_maps = make_in_maps(q, k, v, w_q, w_k, w_v, w_o)
    res = run_bass_kernel_spmd(
        nc,
        in_maps,
        core_ids=list(range(N_CORES)),
        trace=_trace,
        tmpdir=_tmpdir,
    )
    out, attn = gather_outputs(res.results)
    kernel.last_results = res
    return out, attn


# revision 20
# speedup vs baseline: 1.4593x; 1.2203x over previous
"""
MultiHeadAttention forward for Trainium2, 8 NeuronCores.

Sharding across the 8 cores: core c handles batch b = c//4 and the 4
heads h0 = (c%4)*4 .. h0+4 (an E=256 slice of the projection dim).
w_q/w_k/w_v are row-sliced, w_o is column-sliced (Megatron), so each
core emits its attn block plus a partial `out` that the host sums over
the 4 cores sharing a batch.

All inputs are pre-transposed on the host so the kernel never has to
transpose its DRAM inputs:
    qT/kT/vT : [D, S]   (x[b].T)
    wqT/wkT/wvT : [D, E]   (w[e0:e0+E, :].T)  - matmul lhsT for projections
    woT : [E, D]           (w_o[:, e0:e0+E].T) - matmul rhs for out proj

All matmuls run as float32r (single-pass fp32 on the PE, ~4x faster
than the fp32 LOW_HIGH double-pass) with fp32 PSUM accumulation.

Per-core pipeline:
  stage 0: QT[e,s], KT[e,s] (head pairs stacked on the 128 partitions)
           and V[s,e] via PE matmuls.
  stage 1, per head:
    phase A (attn output + denominators), per 128-row tile i:
        S_i = QhT_i.T @ KhT   (PE, K=64, two 2-bank psum halves)
        P_i = exp(S_i/8) with fused row-sum accumulators (ACT)
        invd = 1/den (DVE) ; A_i = P_i * invd in place (DVE, 2x mode)
        DMA A_i -> attn[h, i]
    phase B (context, transposed orientation), per 128-row tile j:
        St_j = KhT_j.T @ QhT  (PE)  ;  Pt_j = exp(St_j/8)  (ACT)
        Ct[hd, :] += Vh_j.T @ Pt_j  (PE, psum accumulate over j)
    C-normalize: invd columns -> PE-transposed into a [1, S] row,
        outer-product broadcast to [64, S], Ct * bcast on DVE.
  stage 2: out = sum over head pairs  Ct_hp.T @ woT_hp -> partial out
"""

import numpy as np

import concourse.bass as bass
import concourse.mybir as mybir
import concourse.tile as tile
from concourse import bacc
from concourse.masks import make_identity

F32 = mybir.dt.float32
F32R = mybir.dt.float32r
BF16 = mybir.dt.bfloat16
MM_DT = BF16  # dtype of all matmul operands (BF16 or F32R)

# Problem shapes (hardcoded per contract)
B, S, D, H = 2, 2048, 1024, 16
HD = D // H                    # 64
N_CORES = 8
H_LOC = H * B // N_CORES       # 4 heads per core
E = H_LOC * HD                 # 256 projection cols per core
P = 128

SCALE = 1.0 / float(np.sqrt(HD))


def _ceil_div(a, b):
    return (a + b - 1) // b


def build_nc(s=S, d=D, mm_dt=MM_DT):
    """Build the single-core Bass program (SPMD across 8 cores)."""
    nt = s // P                  # 128-row tiles of the sequence
    dt_n = d // P                # contraction tiles for projections
    n_ch = _ceil_div(s, 512)     # 512-wide column chunks of the seq dim
    half = min(s, 1024)          # psum half-tile width for scores
    n_hh = _ceil_div(s, half)
    hch = _ceil_div(half, 512)   # 512-chunks per half

    nc = bacc.Bacc("TRN2", target_bir_lowering=False, debug=False)

    def mm(out, lhsT, rhs, heat=False, **kw):
        nc.tensor.matmul(out, lhsT, rhs, **kw)

    qT = nc.dram_tensor("qT", [d, s], mm_dt, kind="ExternalInput")
    kT = nc.dram_tensor("kT", [d, s], mm_dt, kind="ExternalInput")
    vT = nc.dram_tensor("vT", [d, s], mm_dt, kind="ExternalInput")
    wqT = nc.dram_tensor("wqT", [d, E], mm_dt, kind="ExternalInput")
    wkT = nc.dram_tensor("wkT", [d, E], mm_dt, kind="ExternalInput")
    wvT = nc.dram_tensor("wvT", [d, E], mm_dt, kind="ExternalInput")
    woT = nc.dram_tensor("woT", [E, d], mm_dt, kind="ExternalInput")

    attn = nc.dram_tensor("attn", [H_LOC, s, s], F32, kind="ExternalOutput")
    outp = nc.dram_tensor("outp", [s, d], F32, kind="ExternalOutput")

    with tile.TileContext(nc) as tc:
        with (
            tc.tile_pool(name="consts", bufs=1) as consts,
            tc.tile_pool(name="weights", bufs=1) as wpool,
            tc.tile_pool(name="persist", bufs=1) as persist,
            tc.tile_pool(name="work", bufs=2) as work,
            tc.tile_pool(name="stats", bufs=4) as stats,
        ):
            ident = consts.tile([P, P], F32, name="ident")
            make_identity(nc, ident)
            ones_f32 = consts.tile([1, HD], F32, name="ones_f32")
            nc.vector.memset(ones_f32, 1.0)
            ones_row = consts.tile([1, HD], mm_dt, name="ones_row")
            nc.vector.tensor_copy(ones_row, ones_f32)

            # ---- weights to SBUF ----
            wq_sb = wpool.tile([P, dt_n, E], mm_dt, name="wq_sb")
            wk_sb = wpool.tile([P, dt_n, E], mm_dt, name="wk_sb")
            wv_sb = wpool.tile([P, dt_n, E], mm_dt, name="wv_sb")
            wo_sb = wpool.tile([P, E // P, d], mm_dt, name="wo_sb")
            nc.sync.dma_start(wq_sb, wqT.ap().rearrange("(t p) e -> p t e", p=P))
            nc.sync.dma_start(wk_sb, wkT.ap().rearrange("(t p) e -> p t e", p=P))
            nc.sync.dma_start(wv_sb, wvT.ap().rearrange("(t p) e -> p t e", p=P))
            nc.sync.dma_start(wo_sb, woT.ap().rearrange("(t p) f -> p t f", p=P))

            # ---- persistent activations ----
            QT = persist.tile([P, E // P, s], mm_dt, name="QT")
            KT = persist.tile([P, E // P, s], mm_dt, name="KT")
            V_sb = persist.tile([P, nt, E], mm_dt, name="V_sb")
            Ct = persist.tile([P, E // P, s], mm_dt, name="Ct")

            # ================= stage 0: projections =================
            with tc.tile_pool(name="psum0", bufs=2, space="PSUM") as psum0:
                for (src, w_sb, dst) in ((qT, wq_sb, QT), (kT, wk_sb, KT)):
                    ps = [
                        psum0.tile([P, s], F32, name=f"proj_ps{et}", tag="proj")
                        for et in range(E // P)
                    ]
                    for dti in range(dt_n):
                        xin = work.tile([P, s], mm_dt, name="xin", tag="xin", bufs=2)
                        nc.sync.dma_start(
                            xin, src.ap()[dti * P : (dti + 1) * P, :]
                        )
                        for et in range(E // P):
                            for c in range(n_ch):
                                sl = slice(c * 512, min(s, (c + 1) * 512))
                                mm(
                                    ps[et][:, sl],
                                    w_sb[:, dti, et * P : (et + 1) * P],
                                    xin[:, sl],
                                    heat=(et == 0 and c == 0),
                                    start=(dti == 0),
                                    stop=(dti == dt_n - 1),
                                )
                    for et in range(E // P):
                        nc.vector.tensor_copy(dst[:, et, :], ps[et])

                # V natural: one s-tile per 512-wide psum bank
                n_vg = _ceil_div(nt, 4)
                for g in range(n_vg):
                    t0 = g * 4
                    tn = min(4, nt - t0)
                    vps = psum0.tile([P, tn * 512], F32, name="vps", tag="proj")
                    for dti in range(dt_n):
                        xin = work.tile([P, s], mm_dt, name="xin", tag="xin", bufs=2)
                        nc.sync.dma_start(
                            xin, vT.ap()[dti * P : (dti + 1) * P, :]
                        )
                        for t in range(tn):
                            st = t0 + t
                            mm(
                                vps[:, t * 512 : t * 512 + E],
                                xin[:, st * P : (st + 1) * P],
                                wv_sb[:, dti, :],
                                heat=(t == 0),
                                start=(dti == 0),
                                stop=(dti == dt_n - 1),
                            )
                    for t in range(tn):
                        nc.vector.tensor_copy(
                            V_sb[:, t0 + t, :], vps[:, t * 512 : t * 512 + E]
                        )

            # ================= stage 1: attention =================
            # Heads are processed in pairs (2*hp, 2*hp+1), stacked on
            # partitions 0-63 / 64-127 of the QT/KT head-pair tiles. The
            # K=64 score matmuls of a pair land on PE row-groups 0/64 and
            # the M=64 ctx matmuls on column-groups 0/64 (tile_position is
            # derived from the AP base partitions), so each pair executes
            # concurrently on the PE array.
            for hp in range(E // P):
                qh = [QT[x * HD : (x + 1) * HD, hp, :] for x in range(2)]
                kh = [KT[x * HD : (x + 1) * HD, hp, :] for x in range(2)]
                invd = [
                    stats.tile(
                        [P, nt], F32, name=f"invd{x}", tag=f"invd{x}", bufs=2
                    )
                    for x in range(2)
                ]

                # --- phase A: normalized attn rows + denominators ---
                with tc.tile_pool(
                    name=f"psA{hp}", bufs=4, space="PSUM"
                ) as psA:
                    for it in range(nt):
                        a_t = [
                            work.tile([P, s], F32, name=f"a_{x}", tag="A", bufs=3)
                            for x in range(2)
                        ]
                        dens = [
                            stats.tile(
                                [P, max(n_hh, 2)], F32, name=f"dens{x}", tag="dens"
                            )
                            for x in range(2)
                        ]
                        for hh in range(n_hh):
                            w = min(half, s - hh * half)
                            sp2 = [
                                psA.tile(
                                    [P, half], F32, name=f"spA{x}", tag="halfA"
                                )
                                for x in range(2)
                            ]
                            for c in range(hch):
                                lo = hh * half + c * 512
                                if lo >= s:
                                    break
                                wc = min(512, s - lo)
                                for x in range(2):
                                    mm(
                                        sp2[x][:, c * 512 : c * 512 + wc],
                                        qh[x][:, it * P : (it + 1) * P],
                                        kh[x][:, lo : lo + wc],
                                        start=True,
                                        stop=True,
                                    )
                            for x in range(2):
                                nc.scalar.activation(
                                    a_t[x][:, hh * half : hh * half + w],
                                    sp2[x][:, :w],
                                    mybir.ActivationFunctionType.Exp,
                                    scale=SCALE,
                                    accum_out=dens[x][:, hh : hh + 1],
                                )
                        for x in range(2):
                            if n_hh > 1:
                                den = stats.tile([P, 1], F32, name="den", tag="den")
                                nc.vector.reduce_sum(
                                    den,
                                    dens[x][:, :n_hh],
                                    axis=mybir.AxisListType.X,
                                )
                            else:
                                den = dens[x][:, 0:1]
                            nc.vector.reciprocal(invd[x][:, it : it + 1], den)
                            nc.vector.tensor_scalar_mul(
                                a_t[x], a_t[x], invd[x][:, it : it + 1]
                            )
                            nc.sync.dma_start(
                                attn.ap()[2 * hp + x, it * P : (it + 1) * P, :],
                                a_t[x],
                            )

                # --- phase B: context in transposed orientation ---
                # ctx psum accumulators live at base partition 0 (matmul
                # dst at base 64 fails the ISA dst-partition check), so
                # the two heads get separate [64, 1024] tiles and the i
                # range is processed in two halves.
                n_ih = _ceil_div(s, 1024)
                ihw_all = min(s, 1024)
                ctu2 = [
                    work.tile([HD, s], F32, name=f"ctu{x}", tag="CTU", bufs=3)
                    for x in range(2)
                ]
                with tc.tile_pool(
                    name=f"psB{hp}", bufs=1, space="PSUM"
                ) as psB:
                    for ih in range(n_ih):
                        ilo = ih * 1024
                        ihw = min(1024, s - ilo)
                        cts = [
                            psB.tile(
                                [HD, ihw_all], F32, name=f"ct{x}",
                                tag=f"ct{x}", bufs=1,
                            )
                            for x in range(2)
                        ]
                        for jt in range(nt):
                            pt2 = [
                                work.tile(
                                    [P, ihw_all], mm_dt, name=f"pt{x}",
                                    tag="PT", bufs=3,
                                )
                                for x in range(2)
                            ]
                            for cc in range(_ceil_div(ihw, 512)):
                                lo = ilo + cc * 512
                                wc = min(512, s - lo)
                                spB = [
                                    psB.tile(
                                        [P, 512], F32, name=f"spB{x}",
                                        tag="halfB", bufs=4,
                                    )
                                    for x in range(2)
                                ]
                                for x in range(2):
                                    mm(
                                        spB[x][:, :wc],
                                        kh[x][:, jt * P : (jt + 1) * P],
                                        qh[x][:, lo : lo + wc],
                                        start=True,
                                        stop=True,
                                    )
                                for x in range(2):
                                    nc.scalar.activation(
                                        pt2[x][:, cc * 512 : cc * 512 + wc],
                                        spB[x][:, :wc],
                                        mybir.ActivationFunctionType.Exp,
                                        scale=SCALE,
                                    )
                            for cc in range(_ceil_div(ihw, 512)):
                                sl = slice(cc * 512, min(ihw, (cc + 1) * 512))
                                for x in range(2):
                                    mm(
                                        cts[x][:, sl],
                                        V_sb[:, jt, (2 * hp + x) * HD
                                             : (2 * hp + x + 1) * HD],
                                        pt2[x][:, sl],
                                        start=(jt == 0),
                                        stop=(jt == nt - 1),
                                    )
                        for x in range(2):
                            nc.vector.tensor_copy(
                                ctu2[x][:, ilo : ilo + ihw], cts[x][:, :ihw]
                            )

                    # --- C-normalize: Ct * (1/den) broadcast along i ---
                    for x in range(2):
                        row_sb = stats.tile(
                            [1, s], mm_dt, name=f"row_sb{x}", tag="rowsb",
                            bufs=1,
                        )
                        for ih in range(n_ih):
                            ilo = ih * 1024
                            ihw = min(1024, s - ilo)
                            row_ps = psB.tile(
                                [1, ihw_all], F32, name="row_ps", tag="ct0"
                            )
                            t_n = ihw // P
                            for t in range(t_n):
                                it = ih * (1024 // P) + t
                                nc.tensor.matmul(
                                    row_ps[0:1, t * P : (t + 1) * P],
                                    lhsT=invd[x][:, it : it + 1],
                                    rhs=ident,
                                    is_transpose=True,
                                    start=(t % 4 == 0),
                                    stop=(t % 4 == 3 or t == t_n - 1),
                                )
                            nc.vector.tensor_copy(
                                row_sb[0:1, ilo : ilo + ihw], row_ps[0:1, :ihw]
                            )
                        bc_sb = work.tile(
                            [HD, s], F32, name="bc_sb", tag="CTU", bufs=3
                        )
                        for ih in range(n_ih):
                            ilo = ih * 1024
                            ihw = min(1024, s - ilo)
                            bc_ps = psB.tile(
                                [HD, ihw_all], F32, name="bc_ps", tag="ct1"
                            )
                            for c in range(_ceil_div(ihw, 512)):
                                sl = slice(c * 512, min(ihw, (c + 1) * 512))
                                lo = ilo + c * 512
                                wc = min(512, s - lo)
                                mm(
                                    bc_ps[:, sl],
                                    ones_row,
                                    row_sb[0:1, lo : lo + wc],
                                    start=True,
                                    stop=True,
                                )
                            nc.vector.tensor_copy(
                                bc_sb[:, ilo : ilo + ihw], bc_ps[:, :ihw]
                            )
                        nc.vector.tensor_tensor(
                            Ct[x * HD : (x + 1) * HD, hp, :],
                            ctu2[x],
                            bc_sb,
                            mybir.AluOpType.mult,
                        )

            # ================= stage 2: output projection =================
            with tc.tile_pool(name="psum2", bufs=2, space="PSUM") as psum2:
                for it in range(nt):
                    o_ps = psum2.tile([P, d], F32, name="o_ps", tag="out")
                    for hpi in range(E // P):
                        for c in range(_ceil_div(d, 512)):
                            sl = slice(c * 512, min(d, (c + 1) * 512))
                            mm(
                                o_ps[:, sl],
                                Ct[:, hpi, it * P : (it + 1) * P],
                                wo_sb[:, hpi, sl],
                                heat=(hpi == 0 and c == 0),
                                start=(hpi == 0),
                                stop=(hpi == E // P - 1),
                            )
                    o_sb = work.tile([P, d], F32, name="o_sb", tag="O", bufs=2)
                    nc.vector.tensor_copy(o_sb, o_ps)
                    nc.sync.dma_start(outp.ap()[it * P : (it + 1) * P, :], o_sb)

    nc.compile()
    return nc


_NC_CACHE = {}


def _get_nc():
    if "nc" not in _NC_CACHE:
        _NC_CACHE["nc"] = build_nc()
    return _NC_CACHE["nc"]


def make_in_maps(q, k, v, w_q, w_k, w_v, w_o):
    import ml_dtypes

    in_np = ml_dtypes.bfloat16 if MM_DT == BF16 else np.float32
    q = np.asarray(q, np.float32)
    k = np.asarray(k, np.float32)
    v = np.asarray(v, np.float32)
    w_q = np.asarray(w_q, np.float32)
    w_k = np.asarray(w_k, np.float32)
    w_v = np.asarray(w_v, np.float32)
    w_o = np.asarray(w_o, np.float32)

    qT = [np.ascontiguousarray(q[b].T) for b in range(B)]
    kT = [np.ascontiguousarray(k[b].T) for b in range(B)]
    vT = [np.ascontiguousarray(v[b].T) for b in range(B)]

    in_maps = []
    for c in range(N_CORES):
        b = c // (N_CORES // B)
        e0 = (c % (N_CORES // B)) * E
        in_maps.append(
            {
                "qT": qT[b],
                "kT": kT[b],
                "vT": vT[b],
                "wqT": np.ascontiguousarray(w_q[e0 : e0 + E, :].T),
                "wkT": np.ascontiguousarray(w_k[e0 : e0 + E, :].T),
                "wvT": np.ascontiguousarray(w_v[e0 : e0 + E, :].T),
                "woT": np.ascontiguousarray(w_o[:, e0 : e0 + E].T),
            }
        )
    if in_np is not np.float32:
        in_maps = [
            {k2: np.ascontiguousarray(v2.astype(in_np)) for k2, v2 in m.items()}
            for m in in_maps
        ]
    return in_maps


def gather_outputs(results):
    attn = np.empty((B, H, S, S), np.float32)
    out = np.zeros((B, S, D), np.float32)
    for c in range(N_CORES):
        b = c // (N_CORES // B)
        h0 = (c % (N_CORES // B)) * H_LOC
        attn[b, h0 : h0 + H_LOC] = results[c]["attn"]
        out[b] += results[c]["outp"]
    return out, attn


def kernel(q, k, v, w_q, w_k, w_v, w_o, _trace=False, _tmpdir=None):
    from concourse.bass_utils import run_bass_kernel_spmd

    nc = _get_nc()
    in_maps = make_in_maps(q, k, v, w_q, w_k, w_v, w_o)
    res = run_bass_kernel_spmd(
        nc,
        in_maps,
        core_ids=list(range(N_CORES)),
        trace=_trace,
        tmpdir=_tmpdir,
    )
    out, attn = gather_outputs(res.results)
    kernel.last_results = res
    return out, attn
